# revision 14
# baseline (speedup 1.0000x reference)
"""Bidirectional 2-layer LSTM (shared weights across directions) on 8 TRN2 cores.

Strategy: pure data-parallel SPMD. Core c = (dir, batch-quarter): dir in {f, r}
(r gets time-reversed input precomputed on CPU), 8 batch rows per core.
Per core, fully local (no collectives):
  phase A: G0 = Wih[0] @ x^T + bias0 for all steps (big matmul, gates^T layout)
  L0: 512-step recurrence, gates^T = G0[t] + Whh[0] @ h^T (weight-stationary,
      bf16 weights for fast-weight-load; f32 PSUM accumulate + f32 eltwise)
  phase C: G1 = Wih[1] @ h0-stream
  L1: recurrence layer 1
  heads: out^T/hid^T/cel^T partials (this dir's half of the concat contraction)
Python side: shard/pack inputs (bf16 cast, transposes), gather per-core
partials, add f/r partials + biases, assemble full outputs.

All SBUF "transposed" layouts pack a [512, n] tensor as [128, 4*n] with
column = n*k + j  (k = 128-row tile index), so k-tile slices are contiguous.
"""

import os
import numpy as np
import ml_dtypes

B, S, D, H, L = 32, 512, 512, 512, 2
NCORES = 8
BL = 8            # batch rows per core
C = 4 * BL        # h/c state columns per core: (k-tile, b)
GC = 16 * BL      # gates^T psum columns: (m-tile, b)
BF16 = ml_dtypes.bfloat16

LAST_EXEC_NS = None


def _build(nc, tile, bass, mybir, unroll):
    from contextlib import ExitStack

    f32 = mybir.dt.float32
    bf16 = mybir.dt.bfloat16
    AF = mybir.ActivationFunctionType

    # ---- kernel I/O ----
    def din(name, shape, dt=bf16):
        return nc.dram_tensor(name, shape, dt, kind="ExternalInput").ap()

    def dout(name, shape, dt=f32):
        return nc.dram_tensor(name, shape, dt, kind="ExternalOutput").ap()

    xt = din("xt", [128, S * C])                 # x^T packed, (s, k, b) cols
    wih = [din(f"wih{l}t", [128, 4 * 2048]) for l in range(L)]
    whh = [din(f"whh{l}t", [128, 4 * 2048]) for l in range(L)]
    bias = [din(f"bias{l}", [128, 16], f32) for l in range(L)]
    h0t = [din(f"h0t{l}", [128, C]) for l in range(L)]
    c0t = [din(f"c0t{l}", [128, C], f32) for l in range(L)]
    woutt = din("woutt", [128, 4 * 512])
    whidt = din("whidt", [128, 4 * 512])
    wcellt = din("wcellt", [128, 4 * 512])

    outt = dout("outt", [4, 128, S, BL])
    hidt = dout("hidt", [4, 128, L, BL])
    celt = dout("celt", [4, 128, L, BL])
    dbg = dout("dbg", [4, 128, C])          # [tap, part, C] debug taps

    with tile.TileContext(nc) as tc, ExitStack() as stk:
        dram = stk.enter_context(tc.tile_pool(name="dram", bufs=1, space="DRAM"))
        persist = stk.enter_context(tc.tile_pool(name="persist", bufs=1))

        g_dram = [dram.tile([128, S, 128], f32, name=f"g{l}", tag=f"g{l}") for l in range(L)]
        hstream = [dram.tile([128, S * C], bf16, name=f"hs{l}", tag=f"hs{l}") for l in range(L)]

        bias_sb = [persist.tile([128, 16], f32, name=f"bias{l}", tag=f"bias{l}") for l in range(L)]
        h_st = [persist.tile([128, C], bf16, name=f"hst{l}", tag=f"hst{l}") for l in range(L)]
        c_st = [persist.tile([128, C], f32, name=f"cst{l}", tag=f"cst{l}") for l in range(L)]
        for l in range(L):
            nc.sync.dma_start(bias_sb[l][:], bias[l][:])
            nc.sync.dma_start(h_st[l][:], h0t[l][:])
            nc.sync.dma_start(c_st[l][:], c0t[l][:])

        # ---------- G projection phase (big matmul over all steps) ----------
        def gproj(src_res, w_sb, bias_t, g_out):
            # src_res: resident [128, S*C] bf16 (cols = (s, k, b))
            # w_sb:    [128, 4*2048] bf16;  g_out: [128, S, 128] f32 dram
            src4 = src_res.rearrange("p (s k b) -> p s k b", k=4, b=BL)
            with ExitStack() as ps:
                ppool = ps.enter_context(tc.tile_pool(name="gp_psum", bufs=4, space="PSUM"))
                opool = ps.enter_context(tc.tile_pool(name="gp_out", bufs=4))
                for n in range(8):            # chunks of 64 steps
                    for m in range(16):       # gate-row tiles
                        psum = ppool.tile([128, 64 * BL], f32)
                        for k in range(4):
                            nc.tensor.matmul(
                                psum[:],
                                w_sb[:, 2048 * k + 128 * m : 2048 * k + 128 * (m + 1)],
                                src4[:, 64 * n : 64 * (n + 1), k, :],
                                start=(k == 0), stop=(k == 3),
                            )
                        gout = opool.tile([128, 64 * BL], f32)
                        nc.scalar.activation(gout[:], psum[:], AF.Identity,
                                             bias=bias_t[:, m : m + 1])
                        nc.sync.dma_start(
                            g_out[:, 64 * n : 64 * (n + 1), 8 * m : 8 * (m + 1)],
                            gout[:],
                        )

        # ---------- recurrence phase ----------
        def recurrence(l, w_sb):
            g_out = g_dram[l]
            hs, cs = h_st[l], c_st[l]
            with ExitStack() as ps:
                ppool = ps.enter_context(tc.tile_pool(name="rc_psum", bufs=2, space="PSUM"))
                gpool = ps.enter_context(tc.tile_pool(name="rc_g", bufs=3))
                ep = ps.enter_context(tc.tile_pool(name="rc_elt", bufs=2))

                def step(t):
                    g_sb = gpool.tile([128, GC], f32, tag="gin")
                    nc.sync.dma_start(g_sb[:], g_out[:, bass.ds(t, 1), :])
                    psum = ppool.tile([128, GC], f32)
                    for m in range(16):
                        for k in range(4):
                            nc.tensor.matmul(
                                psum[:, 8 * m : 8 * (m + 1)],
                                w_sb[:, 2048 * k + 128 * m : 2048 * k + 128 * (m + 1)],
                                hs[:, 8 * k : 8 * (k + 1)],
                                start=(k == 0), stop=(k == 3),
                            )
                    gates = ep.tile([128, GC], f32, tag="gates")
                    nc.vector.tensor_add(gates[:], psum[:], g_sb[:])
                    sig_if = ep.tile([128, 2 * C], f32, tag="sig_if")
                    nc.scalar.activation(sig_if[:], gates[:, 0 : 2 * C], AF.Sigmoid)
                    tanh_g = ep.tile([128, C], f32, tag="tanh_g")
                    nc.scalar.activation(tanh_g[:], gates[:, 2 * C : 3 * C], AF.Tanh)
                    sig_o = ep.tile([128, C], f32, tag="sig_o")
                    nc.scalar.activation(sig_o[:], gates[:, 3 * C : 4 * C], AF.Sigmoid)
                    t1 = ep.tile([128, C], f32, tag="t1")
                    nc.vector.tensor_mul(t1[:], sig_if[:, C : 2 * C], cs[:])
                    t2 = ep.tile([128, C], f32, tag="t2")
                    nc.vector.tensor_mul(t2[:], sig_if[:, 0:C], tanh_g[:])
                    nc.vector.tensor_add(cs[:], t1[:], t2[:])
                    tanh_c = ep.tile([128, C], f32, tag="tanh_c")
                    nc.scalar.activation(tanh_c[:], cs[:], AF.Tanh)
                    hf = ep.tile([128, C], f32, tag="hf")
                    nc.vector.tensor_mul(hf[:], sig_o[:], tanh_c[:])
                    nc.vector.tensor_copy(hs[:], hf[:])          # f32 -> bf16 cast
                    nc.sync.dma_start(hstream[l][:, bass.ts(t, C)], hs[:])

                tc.For_i_unrolled(0, S, 1, step, max_unroll=unroll)

        # ---------- head matmuls ----------
        def heads(h1_res):
            h14 = h1_res.rearrange("p (s k b) -> p s k b", k=4, b=BL)
            with ExitStack() as ps:
                wpool = ps.enter_context(tc.tile_pool(name="hd_w", bufs=1))
                ppool = ps.enter_context(tc.tile_pool(name="hd_psum", bufs=4, space="PSUM"))
                opool = ps.enter_context(tc.tile_pool(name="hd_out", bufs=4))
                wo = wpool.tile([128, 2048], bf16, tag="wo")
                nc.sync.dma_start(wo[:], woutt[:])
                for n in range(8):
                    for m in range(4):
                        psum = ppool.tile([128, 64 * BL], f32, tag="ps_o")
                        for k in range(4):
                            nc.tensor.matmul(
                                psum[:],
                                wo[:, 512 * k + 128 * m : 512 * k + 128 * (m + 1)],
                                h14[:, 64 * n : 64 * (n + 1), k, :],
                                start=(k == 0), stop=(k == 3),
                            )
                        ot = opool.tile([128, 64 * BL], f32, tag="ot")
                        nc.vector.tensor_copy(ot[:], psum[:])
                        nc.sync.dma_start(outt[m, :, 64 * n : 64 * (n + 1), :], ot[:])

                # hid / cel heads from final states
                cb = [wpool.tile([128, C], bf16, name=f"cbt{l}", tag=f"cbt{l}") for l in range(L)]
                for l in range(L):
                    nc.vector.tensor_copy(cb[l][:], c_st[l][:])
                for name, w_ap, st, out_ap in (
                    ("hid", whidt, h_st, hidt),
                    ("cel", wcellt, cb, celt),
                ):
                    wsb = wpool.tile([128, 2048], bf16, tag=f"w{name}")
                    nc.sync.dma_start(wsb[:], w_ap[:])
                    for m in range(4):
                        ot = opool.tile([128, L * BL], f32, tag=f"ot_{name}")
                        for l in range(L):
                            psum = ppool.tile([128, BL], f32, tag="ps_small")
                            for k in range(4):
                                nc.tensor.matmul(
                                    psum[:],
                                    wsb[:, 512 * k + 128 * m : 512 * k + 128 * (m + 1)],
                                    st[l][:, 8 * k : 8 * (k + 1)],
                                    start=(k == 0), stop=(k == 3),
                                )
                            nc.vector.tensor_copy(ot[:, BL * l : BL * (l + 1)], psum[:])
                        nc.sync.dma_start(out_ap[m, :, :, :], ot[:])

        # ---------- program ----------
        with ExitStack() as ph:
            rp = ph.enter_context(tc.tile_pool(name="ph_a", bufs=1))
            xres = rp.tile([128, S * C], bf16, tag="xres")
            w0 = rp.tile([128, 4 * 2048], bf16, tag="wih0")
            nc.sync.dma_start(xres[:], xt[:])
            nc.sync.dma_start(w0[:], wih[0][:])
            gproj(xres, w0, bias_sb[0], g_dram[0])
        with ExitStack() as ph:
            rp = ph.enter_context(tc.tile_pool(name="ph_l0", bufs=1))
            w = rp.tile([128, 4 * 2048], bf16, tag="whh0")
            nc.sync.dma_start(w[:], whh[0][:])
            recurrence(0, w)
        htmp0 = persist.tile([128, C], f32, name="htmp0", tag="htmp0")
        nc.vector.tensor_copy(htmp0[:], h_st[0][:])
        ctmp0 = persist.tile([128, C], f32, name="ctmp0", tag="ctmp0")
        nc.vector.tensor_copy(ctmp0[:], c_st[0][:])
        nc.gpsimd.dma_start(dbg[0, :, :], htmp0[:])   # h_st[0] right after L0
        nc.gpsimd.dma_start(dbg[2, :, :], ctmp0[:])
        with ExitStack() as ph:
            rp = ph.enter_context(tc.tile_pool(name="ph_c", bufs=1))
            h0res = rp.tile([128, S * C], bf16, tag="h0res")
            w1 = rp.tile([128, 4 * 2048], bf16, tag="wih1")
            nc.sync.dma_start(h0res[:], hstream[0][:])
            nc.sync.dma_start(w1[:], wih[1][:])
            gproj(h0res, w1, bias_sb[1], g_dram[1])
        with ExitStack() as ph:
            rp = ph.enter_context(tc.tile_pool(name="ph_l1", bufs=1))
            w = rp.tile([128, 4 * 2048], bf16, tag="whh1")
            nc.sync.dma_start(w[:], whh[1][:])
            recurrence(1, w)
        htmp1 = persist.tile([128, C], f32, name="htmp1", tag="htmp1")
        nc.vector.tensor_copy(htmp1[:], h_st[0][:])
        ctmp1 = persist.tile([128, C], f32, name="ctmp1", tag="ctmp1")
        nc.vector.tensor_copy(ctmp1[:], c_st[0][:])
        nc.gpsimd.dma_start(dbg[1, :, :], htmp1[:])   # h_st[0] right before heads
        nc.gpsimd.dma_start(dbg[3, :, :], ctmp1[:])
        with ExitStack() as ph:
            rp = ph.enter_context(tc.tile_pool(name="ph_hd", bufs=1))
            h1res = rp.tile([128, S * C], bf16, tag="h1res")
            nc.sync.dma_start(h1res[:], hstream[1][:])
            heads(h1res)


def _pack_T(w):
    # [rows, 512] -> [128, 4*rows] with col = rows*k + r
    rows = w.shape[0]
    return np.ascontiguousarray(
        w.T.reshape(4, 128, rows).transpose(1, 0, 2).reshape(128, 4 * rows)
    )


def _pack_state(v):
    # [BL, 512] -> [128, 32] with col = 8*k + b
    return np.ascontiguousarray(
        v.T.reshape(4, 128, BL).transpose(1, 0, 2).reshape(128, C)
    )


def _prep_core(dir_, q, I):
    bs = slice(q * BL, (q + 1) * BL)
    x = I["inputs"][bs]                      # [BL, S, D]
    if dir_ == 1:
        x = x[:, ::-1]
    a = x.transpose(1, 2, 0).reshape(S, 4, 128, BL).transpose(2, 0, 1, 3)
    m = {"xt": np.ascontiguousarray(a.reshape(128, S * C)).astype(BF16)}
    for l in range(L):
        m[f"wih{l}t"] = _pack_T(I["Wih"][l]).astype(BF16)
        m[f"whh{l}t"] = _pack_T(I["Whh"][l]).astype(BF16)
        m[f"bias{l}"] = np.ascontiguousarray(
            (I["bih"][l] + I["bhh"][l]).reshape(16, 128).T
        ).astype(np.float32)
        m[f"h0t{l}"] = _pack_state(I["h0"][l, bs]).astype(BF16)
        m[f"c0t{l}"] = _pack_state(I["c0"][l, bs]).astype(np.float32)
    half = slice(dir_ * 512, (dir_ + 1) * 512)
    m["woutt"] = _pack_T(I["Wout"][:, half]).astype(BF16)
    m["whidt"] = _pack_T(I["Whid"][:, half]).astype(BF16)
    m["wcellt"] = _pack_T(I["Wcell"][:, half]).astype(BF16)
    return m


def kernel(**inputs):
    global LAST_EXEC_NS
    import concourse.bass as bass
    import concourse.mybir as mybir
    import concourse.tile as tile
    from concourse import bacc
    from concourse.bass_utils import run_bass_kernel_spmd

    I = {k: np.asarray(v, dtype=np.float32) for k, v in inputs.items()}

    unroll = int(os.environ.get("KERNEL_UNROLL", "4"))
    nc = bacc.Bacc("TRN2", target_bir_lowering=False, debug=False,
                   enable_asserts=False, num_devices=NCORES)
    _build(nc, tile, bass, mybir, unroll)
    nc.compile()

    in_maps = [_prep_core(c // 4, c % 4, I) for c in range(NCORES)]
    trace = os.environ.get("KERNEL_PROFILE", "0") == "1"
    import time as _time
    t0 = _time.time()
    res = run_bass_kernel_spmd(nc, in_maps, core_ids=list(range(NCORES)),
                               trace=trace)
    wall_ns = int((_time.time() - t0) * 1e9)
    LAST_EXEC_NS = res.exec_time_ns if res.exec_time_ns else wall_ns
    kernel.RAW = res.results

    output = np.zeros((B, S, 512), np.float32)
    hid = np.zeros((L, B, 512), np.float32)
    cel = np.zeros((L, B, 512), np.float32)
    for c in range(NCORES):
        dir_, q = c // 4, c % 4
        bs = slice(q * BL, (q + 1) * BL)
        r = res.results[c]
        po = r["outt"].transpose(3, 2, 0, 1).reshape(BL, S, 512)
        if dir_ == 1:
            po = po[:, ::-1]
        output[bs] += po
        hid[:, bs] += r["hidt"].transpose(2, 3, 0, 1).reshape(L, BL, 512)
        cel[:, bs] += r["celt"].transpose(2, 3, 0, 1).reshape(L, BL, 512)
    output += I["bout"]
    hid += I["bhid"]
    cel += I["bcell"]
    return output, hid, cel


# revision 18
# speedup vs baseline: 9.4903x; 9.4903x over previous
"""Bidirectional 2-layer LSTM (shared weights across directions) on 8 TRN2 cores.

Strategy: pure data-parallel SPMD. Core c = (dir, batch-quarter): dir in {f, r}
(r gets time-reversed input precomputed on CPU), 8 batch rows per core.
Per core, fully local (no collectives):
  phase A: G0 = Wih[0] @ x^T + bias0 for all steps (big matmul, gates^T layout)
  L0: 512-step recurrence, gates^T = G0[t] + Whh[0] @ h^T (weight-stationary,
      bf16 weights for fast-weight-load; f32 PSUM accumulate + f32 eltwise)
  phase C: G1 = Wih[1] @ h0-stream
  L1: recurrence layer 1
  heads: out^T/hid^T/cel^T partials (this dir's half of the concat contraction)
Python side: shard/pack inputs (bf16 cast, transposes), gather per-core
partials, add f/r partials + biases, assemble full outputs.

All SBUF "transposed" layouts pack a [512, n] tensor as [128, 4*n] with
column = n*k + j  (k = 128-row tile index), so k-tile slices are contiguous.
"""

import os
import numpy as np
import ml_dtypes

B, S, D, H, L = 32, 512, 512, 512, 2
NCORES = 8
BL = 8            # batch rows per core
C = 4 * BL        # h/c state columns per core: (k-tile, b)
GC = 16 * BL      # gates^T psum columns: (m-tile, b)
BF16 = ml_dtypes.bfloat16

LAST_EXEC_NS = None


def _build(nc, tile, bass, mybir, unroll):
    from contextlib import ExitStack

    f32 = mybir.dt.float32
    bf16 = mybir.dt.bfloat16
    AF = mybir.ActivationFunctionType

    # ---- kernel I/O ----
    def din(name, shape, dt=bf16):
        return nc.dram_tensor(name, shape, dt, kind="ExternalInput").ap()

    def dout(name, shape, dt=f32):
        return nc.dram_tensor(name, shape, dt, kind="ExternalOutput").ap()

    xt = din("xt", [128, S * C])                 # x^T packed, (s, k, b) cols
    wih = [din(f"wih{l}t", [128, 4 * 2048]) for l in range(L)]
    whh = [din(f"whh{l}t", [128, 4 * 2048]) for l in range(L)]
    bias = [din(f"bias{l}", [128, 16], f32) for l in range(L)]
    h0t = [din(f"h0t{l}", [128, C]) for l in range(L)]
    c0t = [din(f"c0t{l}", [128, C], f32) for l in range(L)]
    woutt = din("woutt", [128, 4 * 512])
    whidt = din("whidt", [128, 4 * 512])
    wcellt = din("wcellt", [128, 4 * 512])

    outt = dout("outt", [4, 128, S, BL])
    hidt = dout("hidt", [4, 128, L, BL])
    celt = dout("celt", [4, 128, L, BL])
    dbg = dout("dbg", [4, 128, C])          # [tap, part, C] debug taps

    with tile.TileContext(nc) as tc, ExitStack() as stk:
        dram = stk.enter_context(tc.tile_pool(name="dram", bufs=1, space="DRAM"))
        persist = stk.enter_context(tc.tile_pool(name="persist", bufs=1))

        g_dram = [dram.tile([128, S, 128], f32, name=f"g{l}", tag=f"g{l}") for l in range(L)]
        hstream = [dram.tile([128, S * C], bf16, name=f"hs{l}", tag=f"hs{l}") for l in range(L)]

        bias_sb = [persist.tile([128, 16], f32, name=f"bias{l}", tag=f"bias{l}") for l in range(L)]
        h_st = [persist.tile([128, C], bf16, name=f"hst{l}", tag=f"hst{l}") for l in range(L)]
        c_st = [persist.tile([128, C], f32, name=f"cst{l}", tag=f"cst{l}") for l in range(L)]
        for l in range(L):
            nc.sync.dma_start(bias_sb[l][:], bias[l][:])
            nc.sync.dma_start(h_st[l][:], h0t[l][:])
            nc.sync.dma_start(c_st[l][:], c0t[l][:])

        # ---------- G projection phase (big matmul over all steps) ----------
        def gproj(src_res, w_sb, bias_t, g_out):
            # src_res: resident [128, S*C] bf16 (cols = (s, k, b))
            # w_sb:    [128, 4*2048] bf16;  g_out: [128, S, 128] f32 dram
            src4 = src_res.rearrange("p (s k b) -> p s k b", k=4, b=BL)
            with ExitStack() as ps:
                ppool = ps.enter_context(tc.tile_pool(name="gp_psum", bufs=4, space="PSUM"))
                opool = ps.enter_context(tc.tile_pool(name="gp_out", bufs=4))
                for n in range(8):            # chunks of 64 steps
                    for m in range(16):       # gate-row tiles
                        psum = ppool.tile([128, 64 * BL], f32)
                        for k in range(4):
                            nc.tensor.matmul(
                                psum[:],
                                w_sb[:, 2048 * k + 128 * m : 2048 * k + 128 * (m + 1)],
                                src4[:, 64 * n : 64 * (n + 1), k, :],
                                start=(k == 0), stop=(k == 3),
                            )
                        gout = opool.tile([128, 64 * BL], f32)
                        nc.scalar.activation(gout[:], psum[:], AF.Identity,
                                             bias=bias_t[:, m : m + 1])
                        nc.sync.dma_start(
                            g_out[:, 64 * n : 64 * (n + 1), 8 * m : 8 * (m + 1)],
                            gout[:],
                        )

        # ---------- recurrence phase ----------
        def recurrence(l, w_sb):
            g_out = g_dram[l]
            hs, cs = h_st[l], c_st[l]
            with ExitStack() as ps:
                ppool = ps.enter_context(tc.tile_pool(name="rc_psum", bufs=2, space="PSUM"))
                gpool = ps.enter_context(tc.tile_pool(name="rc_g", bufs=3))
                ep = ps.enter_context(tc.tile_pool(name="rc_elt", bufs=2))

                def step(t):
                    g_sb = gpool.tile([128, GC], f32, tag="gin")
                    nc.sync.dma_start(g_sb[:], g_out[:, bass.ds(t, 1), :])
                    psum = ppool.tile([128, GC], f32)
                    for m in range(16):
                        for k in range(4):
                            nc.tensor.matmul(
                                psum[:, 8 * m : 8 * (m + 1)],
                                w_sb[:, 2048 * k + 128 * m : 2048 * k + 128 * (m + 1)],
                                hs[:, 8 * k : 8 * (k + 1)],
                                start=(k == 0), stop=(k == 3),
                            )
                    # gate blocks are CPU-reordered to (i, f, o, g):
                    # i=[0:C), f=[C:2C), o=[2C:3C), g=[3C:4C)
                    gates = ep.tile([128, GC], f32, tag="gates")
                    nc.vector.tensor_add(gates[:], psum[:], g_sb[:])
                    sio = ep.tile([128, 3 * C], f32, tag="sio")
                    nc.scalar.activation(sio[:], gates[:, 0 : 3 * C], AF.Sigmoid)
                    tanh_g = ep.tile([128, C], f32, tag="tanh_g")
                    nc.scalar.activation(tanh_g[:], gates[:, 3 * C : 4 * C], AF.Tanh)
                    t1 = ep.tile([128, C], f32, tag="t1")
                    nc.vector.tensor_mul(t1[:], sio[:, C : 2 * C], cs[:])
                    t2 = ep.tile([128, C], f32, tag="t2")
                    nc.vector.tensor_mul(t2[:], sio[:, 0:C], tanh_g[:])
                    nc.vector.tensor_add(cs[:], t1[:], t2[:])
                    tanh_c = ep.tile([128, C], f32, tag="tanh_c")
                    nc.scalar.activation(tanh_c[:], cs[:], AF.Tanh)
                    hf = ep.tile([128, C], f32, tag="hf")
                    nc.vector.tensor_mul(hf[:], sio[:, 2 * C : 3 * C], tanh_c[:])
                    nc.vector.tensor_copy(hs[:], hf[:])          # f32 -> bf16 cast
                    nc.sync.dma_start(hstream[l][:, bass.ts(t, C)], hs[:])

                def unrollable_body(iv0, n_un):
                    for i in range(n_un):
                        step(iv0 + i)

                tc.For_i_unrolled_general(
                    0, S, 1, unrollable_body, max_unroll=unroll,
                    hint_engines=(mybir.EngineType.PE,
                                  mybir.EngineType.DVE,
                                  mybir.EngineType.Activation),
                )

        # ---------- head matmuls ----------
        def heads(h1_res):
            h14 = h1_res.rearrange("p (s k b) -> p s k b", k=4, b=BL)
            with ExitStack() as ps:
                wpool = ps.enter_context(tc.tile_pool(name="hd_w", bufs=1))
                ppool = ps.enter_context(tc.tile_pool(name="hd_psum", bufs=4, space="PSUM"))
                opool = ps.enter_context(tc.tile_pool(name="hd_out", bufs=4))
                wo = wpool.tile([128, 2048], bf16, tag="wo")
                nc.sync.dma_start(wo[:], woutt[:])
                for n in range(8):
                    for m in range(4):
                        psum = ppool.tile([128, 64 * BL], f32, tag="ps_o")
                        for k in range(4):
                            nc.tensor.matmul(
                                psum[:],
                                wo[:, 512 * k + 128 * m : 512 * k + 128 * (m + 1)],
                                h14[:, 64 * n : 64 * (n + 1), k, :],
                                start=(k == 0), stop=(k == 3),
                            )
                        ot = opool.tile([128, 64 * BL], f32, tag="ot")
                        nc.vector.tensor_copy(ot[:], psum[:])
                        nc.sync.dma_start(outt[m, :, 64 * n : 64 * (n + 1), :], ot[:])

                # hid / cel heads from final states
                cb = [wpool.tile([128, C], bf16, name=f"cbt{l}", tag=f"cbt{l}") for l in range(L)]
                for l in range(L):
                    nc.vector.tensor_copy(cb[l][:], c_st[l][:])
                for name, w_ap, st, out_ap in (
                    ("hid", whidt, h_st, hidt),
                    ("cel", wcellt, cb, celt),
                ):
                    wsb = wpool.tile([128, 2048], bf16, tag=f"w{name}")
                    nc.sync.dma_start(wsb[:], w_ap[:])
                    for m in range(4):
                        ot = opool.tile([128, L * BL], f32, tag=f"ot_{name}")
                        for l in range(L):
                            psum = ppool.tile([128, BL], f32, tag="ps_small")
                            for k in range(4):
                                nc.tensor.matmul(
                                    psum[:],
                                    wsb[:, 512 * k + 128 * m : 512 * k + 128 * (m + 1)],
                                    st[l][:, 8 * k : 8 * (k + 1)],
                                    start=(k == 0), stop=(k == 3),
                                )
                            nc.vector.tensor_copy(ot[:, BL * l : BL * (l + 1)], psum[:])
                        nc.sync.dma_start(out_ap[m, :, :, :], ot[:])

        # ---------- program ----------
        with ExitStack() as ph:
            rp = ph.enter_context(tc.tile_pool(name="ph_a", bufs=1))
            xres = rp.tile([128, S * C], bf16, tag="xres")
            w0 = rp.tile([128, 4 * 2048], bf16, tag="wih0")
            nc.sync.dma_start(xres[:], xt[:])
            nc.sync.dma_start(w0[:], wih[0][:])
            gproj(xres, w0, bias_sb[0], g_dram[0])
        with ExitStack() as ph:
            rp = ph.enter_context(tc.tile_pool(name="ph_l0", bufs=1))
            w = rp.tile([128, 4 * 2048], bf16, tag="whh0")
            nc.sync.dma_start(w[:], whh[0][:])
            recurrence(0, w)
        htmp0 = persist.tile([128, C], f32, name="htmp0", tag="htmp0")
        nc.vector.tensor_copy(htmp0[:], h_st[0][:])
        ctmp0 = persist.tile([128, C], f32, name="ctmp0", tag="ctmp0")
        nc.vector.tensor_copy(ctmp0[:], c_st[0][:])
        nc.gpsimd.dma_start(dbg[0, :, :], htmp0[:])   # h_st[0] right after L0
        nc.gpsimd.dma_start(dbg[2, :, :], ctmp0[:])
        with ExitStack() as ph:
            rp = ph.enter_context(tc.tile_pool(name="ph_c", bufs=1))
            h0res = rp.tile([128, S * C], bf16, tag="h0res")
            w1 = rp.tile([128, 4 * 2048], bf16, tag="wih1")
            nc.sync.dma_start(h0res[:], hstream[0][:])
            nc.sync.dma_start(w1[:], wih[1][:])
            gproj(h0res, w1, bias_sb[1], g_dram[1])
        with ExitStack() as ph:
            rp = ph.enter_context(tc.tile_pool(name="ph_l1", bufs=1))
            w = rp.tile([128, 4 * 2048], bf16, tag="whh1")
            nc.sync.dma_start(w[:], whh[1][:])
            recurrence(1, w)
        htmp1 = persist.tile([128, C], f32, name="htmp1", tag="htmp1")
        nc.vector.tensor_copy(htmp1[:], h_st[0][:])
        ctmp1 = persist.tile([128, C], f32, name="ctmp1", tag="ctmp1")
        nc.vector.tensor_copy(ctmp1[:], c_st[0][:])
        nc.gpsimd.dma_start(dbg[1, :, :], htmp1[:])   # h_st[0] right before heads
        nc.gpsimd.dma_start(dbg[3, :, :], ctmp1[:])
        with ExitStack() as ph:
            rp = ph.enter_context(tc.tile_pool(name="ph_hd", bufs=1))
            h1res = rp.tile([128, S * C], bf16, tag="h1res")
            nc.sync.dma_start(h1res[:], hstream[1][:])
            heads(h1res)


def _pack_T(w):
    # [rows, 512] -> [128, 4*rows] with col = rows*k + r
    rows = w.shape[0]
    return np.ascontiguousarray(
        w.T.reshape(4, 128, rows).transpose(1, 0, 2).reshape(128, 4 * rows)
    )


def _pack_state(v):
    # [BL, 512] -> [128, 32] with col = 8*k + b
    return np.ascontiguousarray(
        v.T.reshape(4, 128, BL).transpose(1, 0, 2).reshape(128, C)
    )


def _prep_core(dir_, q, I):
    bs = slice(q * BL, (q + 1) * BL)
    x = I["inputs"][bs]                      # [BL, S, D]
    if dir_ == 1:
        x = x[:, ::-1]
    a = x.transpose(1, 2, 0).reshape(S, 4, 128, BL).transpose(2, 0, 1, 3)
    m = {"xt": np.ascontiguousarray(a.reshape(128, S * C)).astype(BF16)}
    # reorder gate blocks (i,f,g,o) -> (i,f,o,g) so sigmoid covers one slab
    perm = np.r_[0:1024, 1536:2048, 1024:1536]
    for l in range(L):
        m[f"wih{l}t"] = _pack_T(I["Wih"][l][perm]).astype(BF16)
        m[f"whh{l}t"] = _pack_T(I["Whh"][l][perm]).astype(BF16)
        m[f"bias{l}"] = np.ascontiguousarray(
            (I["bih"][l] + I["bhh"][l])[perm].reshape(16, 128).T
        ).astype(np.float32)
        m[f"h0t{l}"] = _pack_state(I["h0"][l, bs]).astype(BF16)
        m[f"c0t{l}"] = _pack_state(I["c0"][l, bs]).astype(np.float32)
    half = slice(dir_ * 512, (dir_ + 1) * 512)
    m["woutt"] = _pack_T(I["Wout"][:, half]).astype(BF16)
    m["whidt"] = _pack_T(I["Whid"][:, half]).astype(BF16)
    m["wcellt"] = _pack_T(I["Wcell"][:, half]).astype(BF16)
    return m


def kernel(**inputs):
    global LAST_EXEC_NS
    import concourse.bass as bass
    import concourse.mybir as mybir
    import concourse.tile as tile
    from concourse import bacc
    from concourse.bass_utils import run_bass_kernel_spmd

    I = {k: np.asarray(v, dtype=np.float32) for k, v in inputs.items()}

    unroll = int(os.environ.get("KERNEL_UNROLL", "4"))
    nc = bacc.Bacc("TRN2", target_bir_lowering=False, debug=False,
                   enable_asserts=False, num_devices=NCORES)
    _build(nc, tile, bass, mybir, unroll)
    nc.compile()

    in_maps = [_prep_core(c // 4, c % 4, I) for c in range(NCORES)]
    trace = os.environ.get("KERNEL_PROFILE", "0") == "1"
    import time as _time
    t0 = _time.time()
    res = run_bass_kernel_spmd(nc, in_maps, core_ids=list(range(NCORES)),
                               trace=trace)
    wall_ns = int((_time.time() - t0) * 1e9)
    LAST_EXEC_NS = res.exec_time_ns if res.exec_time_ns else wall_ns
    kernel.RAW = res.results

    output = np.zeros((B, S, 512), np.float32)
    hid = np.zeros((L, B, 512), np.float32)
    cel = np.zeros((L, B, 512), np.float32)
    for c in range(NCORES):
        dir_, q = c // 4, c % 4
        bs = slice(q * BL, (q + 1) * BL)
        r = res.results[c]
        po = r["outt"].transpose(3, 2, 0, 1).reshape(BL, S, 512)
        if dir_ == 1:
            po = po[:, ::-1]
        output[bs] += po
        hid[:, bs] += r["hidt"].transpose(2, 3, 0, 1).reshape(L, BL, 512)
        cel[:, bs] += r["celt"].transpose(2, 3, 0, 1).reshape(L, BL, 512)
    output += I["bout"]
    hid += I["bhid"]
    cel += I["bcell"]
    return output, hid, cel
